# revision 24
# baseline (speedup 1.0000x reference)
# Trainium2 Bass kernel for nn_CoefficientLayer (per-species MLP dispatch,
# ANI-style).
#
# v4 routing: 65536 atoms / 4 species (~16384 each) -> each species owns
# exactly TWO of the 8 cores (species s -> cores 2s, 2s+1).  The device
# program is species-agnostic: every core runs the identical schedule over
# one species' atoms; the species routing lives entirely in (a) which atom
# rows the host packs for each core and (b) the per-core weight image passed
# via in_maps.  The shifter slope s1 is folded into W4 host-side so even the
# output stage is species-independent (scale=1.0, per-core bias column).
#
# Device pipeline per 512-atom tile (all-bf16 datapath, fp32 psum), with
# tile_position packing: L1/L2's two K=1 bias ("aug") matmuls run
# concurrently in row groups 0/32 (different PSUM banks); L3-m1 (M=32, col
# group 64) runs concurrently with L4 (M=1, col group 0) of the previous
# pipeline stage.  16 PE slots per tile.
#
# Math (e = exp(1)): stored Hb_k = e*(elu(y_k)+1), y_k = x_k/alpha, via
#   e*(elu(y)+1) = max(e*(y+1), min(exp(y+1), e))
#   psum = e*(y+1) from scaled weight chunks + augmented bias rows
#   ACT:  E = Exp(psum/e);   DVE:  Hb = (E min e) max psum
# Zero-padded weight columns make pad rows of Hb exactly 1.0 (feeds L3's
# aug row riding k1's row 64; killed by zero rows in the next lhsT).
# L4: psum4 = (s1*alpha/e)*W4^T Hb3, out = psum4 + (s0 + s1*alpha*beta4).
import numpy as np
from contextlib import ExitStack

import concourse.bass as bass
import concourse.tile as tile
from concourse import bacc, mybir
from concourse.bass_utils import run_bass_kernel_spmd

ALPHA = 0.1
E1 = float(np.exp(1.0))
P = 128
NCORES = 8
QUANTUM = 128
DIN = 384
DIMS = [384, 256, 192, 160]

F32 = mybir.dt.float32
BF16 = mybir.dt.bfloat16
AF = mybir.ActivationFunctionType
ALU = mybir.AluOpType

# layer -> (n_weight_chunks_per_m (incl aug), n_m_chunks, true_K, true_M)
CHUNKS = {1: (4, 2, 384, 256), 2: (3, 2, 256, 192), 3: (2, 2, 192, 160),
          4: (2, 1, 160, 1)}
WCOLS = (8 + 6 + 4) * P + 2


def _wcol(layer, m, k):
    off = 0
    for l in (1, 2, 3):
        nk, nm = CHUNKS[l][0], CHUNKS[l][1]
        if l == layer:
            return off + (m * nk + k) * P
        off += nk * nm * P
    assert layer == 4 and m == 0
    return off + k


def _fold_host(inputs):
    """Per-species weight image [4][128, WCOLS] + shifter bias."""
    al = ALPHA
    wimgs, bimgs = [], []
    for s in range(4):
        W = [np.asarray(inputs[f"W{i}"][s], np.float32) for i in (1, 2, 3, 4)]
        b = [np.asarray(inputs[f"b{i}"][s], np.float32) for i in (1, 2, 3, 4)]
        s1 = float(np.asarray(inputs["shift_b1"], np.float32)[s])
        s0 = float(np.asarray(inputs["shift_b0"], np.float32)[s])
        Wt = [(E1 / al) * W[0], W[1], W[2], (s1 * al / E1) * W[3]]
        aug = [E1 * (b[0] / al + 1.0),
               E1 * (b[1] / al - W[1].sum(axis=0) + 1.0),
               E1 * (b[2] / al - W[2].sum(axis=0) + 1.0)]
        beta4 = b[3] - al * W[3].sum(axis=0)

        wimg = np.zeros((P, WCOLS), dtype=np.float32)
        for layer in (1, 2, 3, 4):
            nk, nm, tk, tm = CHUNKS[layer]
            Wl = Wt[layer - 1]
            w = 1 if layer == 4 else P
            for m in range(nm):
                mlo, mhi = m * P, min((m + 1) * P, tm)
                for k in range(nk):
                    blk = np.zeros((P, w), np.float32)
                    is_aug = (layer in (1, 2)) and (k == nk - 1)
                    if is_aug:
                        # aug row at the partition matching its packed row
                        # position: m0 -> row 0, m1 -> row 32
                        blk[32 * m, :mhi - mlo] = aug[layer - 1][mlo:mhi]
                    else:
                        rows = Wl[k * P:min((k + 1) * P, tk), mlo:mhi]
                        if layer == 4 and k == 1:
                            # W4's K-tail at partitions 64:96 so L4-k1 runs
                            # in row group 64 beside L3-m1's output slice
                            blk[64:64 + rows.shape[0], :rows.shape[1]] = rows
                        else:
                            blk[:rows.shape[0], :rows.shape[1]] = rows
                        if layer == 3 and k == 1:
                            # aug rides the zero-pad row 64 (Hb2m1 pad = 1)
                            blk[64, :mhi - mlo] = aug[2][mlo:mhi]
                    wimg[:, _wcol(layer, m, k):_wcol(layer, m, k) + w] = blk
        wimgs.append(wimg)
        bimgs.append(np.full((P, 1), s0 + s1 * float(beta4[0]), np.float32))
    return wimgs, bimgs


def _make_sched(A_pc):
    """Tile list (col, n): smallest tiles at the schedule edges."""
    tiles = []
    rem, col = A_pc, 0
    while rem > 0:
        if rem >= 512:
            n = 512
        else:
            n = rem
        assert n >= 256 or rem == n, (rem, n)
        tiles.append((col, n))
        col += n
        rem -= n
    # smallest tile LAST (shortest drain chains); second-smallest first
    tiles.sort(key=lambda t: -t[1])
    if tiles[-1][1] != 512 and len(tiles) >= 2:
        small = tiles[-1]
        rest = tiles[:-1]
        if rest[-1][1] != 512:
            lead, mid = [rest[-1]], rest[:-1]
        else:
            lead, mid = [], rest
        return lead + mid + [small]
    return tiles


def _host_prepare(inputs):
    species = np.asarray(inputs["species"]).ravel()
    aev = np.ascontiguousarray(np.asarray(inputs["aev"], np.float32).reshape(-1, DIN))
    order = np.argsort(species, kind="stable")
    counts = np.bincount(species, minlength=4)
    # species s -> cores 2s, 2s+1 (half the species' atoms each)
    loads = []
    for s in range(4):
        h = (int(counts[s]) + 1) // 2
        loads += [h, int(counts[s]) - h]
    A_pc = max(512, int(np.ceil(max(loads) / QUANTUM)) * QUANTUM)

    idx = np.full((NCORES, A_pc), -1, dtype=np.int64)
    off = 0
    for s in range(4):
        grp = order[off:off + counts[s]]
        off += counts[s]
        h = (int(counts[s]) + 1) // 2
        idx[2 * s, :h] = grp[:h]
        idx[2 * s + 1, :counts[s] - h] = grp[h:]

    import ml_dtypes
    aev_t = np.zeros((NCORES, DIN, A_pc), dtype=ml_dtypes.bfloat16)
    for c in range(NCORES):
        valid = idx[c] >= 0
        aev_t[c][:, valid] = aev[idx[c][valid]].T.astype(ml_dtypes.bfloat16)

    sched = _make_sched(A_pc)
    return aev_t, idx, sched, A_pc


def _build_program(sched, A_pc):
    nc = bacc.Bacc("TRN2", target_bir_lowering=False, debug=False)
    aev_d = nc.dram_tensor("aev_t", [DIN, A_pc], BF16, kind="ExternalInput").ap()
    w_d = nc.dram_tensor("wimg", [P, WCOLS], BF16, kind="ExternalInput").ap()
    b_d = nc.dram_tensor("bimg", [P, 1], F32, kind="ExternalInput").ap()
    out_d = nc.dram_tensor("out", [1, A_pc], F32, kind="ExternalOutput").ap()

    with tile.TileContext(nc) as tc, ExitStack() as ctx:
        wpool = ctx.enter_context(tc.tile_pool(name="w", bufs=1))
        xpool = ctx.enter_context(tc.tile_pool(name="x", bufs=5))
        hpool = ctx.enter_context(tc.tile_pool(name="h", bufs=4))
        epool = ctx.enter_context(tc.tile_pool(name="e", bufs=4))

        pspool = ctx.enter_context(tc.tile_pool(name="ps", bufs=1, space="PSUM"))
        ps4pool = ctx.enter_context(tc.tile_pool(name="ps4", bufs=2, space="PSUM"))

        lay_cols = {1: 8 * P, 2: 6 * P, 3: 4 * P, 4: 2}
        lay_off = {1: 0, 2: 8 * P, 3: 14 * P, 4: 18 * P}
        wtiles = {}

        def load_weights(layers):
            for ly in layers:
                wt = wpool.tile([P, lay_cols[ly]], BF16, tag=f"wL{ly}")
                o = lay_off[ly]
                if ly == 1:  # m0's k-chunks first: first matmul starts sooner
                    nc.sync.dma_start(wt[:, :3 * P], w_d[:, o:o + 3 * P])
                    nc.sync.dma_start(wt[:, 3 * P:], w_d[:, o + 3 * P:o + lay_cols[ly]])
                else:
                    nc.sync.dma_start(wt[:], w_d[:, o:o + lay_cols[ly]])
                wtiles[ly] = wt

        load_weights((1,))
        xloads = {}

        def stage_load(t, split=False):
            col, n = sched[t]
            xt = xpool.tile([P, 3, 512], BF16, tag="x")
            src = aev_d.rearrange("(k p) a -> p k a", k=3)
            if split:
                for k in range(3):
                    nc.sync.dma_start(xt[:, k, :n], src[:, k, col:col + n])
            else:
                nc.sync.dma_start(xt[:, :, :n], src[:, :, col:col + n])
            xloads[t] = [xt[:, k, :n] for k in range(3)]

        stage_load(0, split=True)
        bsb = wpool.tile([P, 1], F32, tag="bimg")
        nc.sync.dma_start(bsb[:], b_d[:])
        ystage = wpool.tile([1, A_pc], F32, tag="ystage")
        ones = wpool.tile([P, 512], BF16, tag="ones")
        nc.vector.memset(ones[:], 1.0)
        # touch Exp during the DMA prologue so the ACT table load is off the
        # first tile's critical path
        scratch = wpool.tile([1, 1], F32, tag="scratch")
        nc.scalar.activation(scratch[:], ones[0:1, 0:1], AF.Exp,
                             bias=0.0, scale=1.0)
        if len(sched) > 1:
            stage_load(1)  # before the deferred L2-4 weight DMAs
        warm = pspool.tile([P, 2, 512], F32, tag="ps1")
        for _ in range(7):
            nc.tensor.matmul(warm[:, 0, :], ones[:, 0:128], ones[:],
                             start=True, stop=True)

        def wsl(layer, m, k, width=P):
            c0 = _wcol(layer, m, k) - lay_off[layer]
            return wtiles[layer][:, c0:c0 + width]

        T = len(sched)
        hid = {}

        def post(ps, t, layer, n):
            et = epool.tile([P, 2, 512], BF16, tag="e")
            nc.scalar.activation(et[:, :, :n], ps[:, :, :n], AF.Exp,
                                 bias=0.0, scale=1.0 / E1)
            ht = hpool.tile([P, 2, 512], BF16, tag=f"h{layer}")
            nc.vector.scalar_tensor_tensor(
                ht[:, :, :n], et[:, :, :n], E1, ps[:, :, :n],
                ALU.min, ALU.max)
            hid[(t, layer)] = ht

        def stage_l1(t):
            col, n = sched[t]
            hs = xloads.pop(t)
            # warmup: first two L1 tiles borrow the not-yet-used ps3/ps2
            # banks so three L1s can run back-to-back while their Exp->stt
            # chains complete
            tag = "ps3" if t == 0 else ("ps2" if t == 1 else "ps1")
            ps = pspool.tile([P, 2, 512], F32, tag=tag)
            for m in range(2):
                for k in range(3):
                    nc.tensor.matmul(ps[:, m, :n], wsl(1, m, k), hs[k],
                                     start=(k == 0), stop=False)
            nc.tensor.matmul(ps[:, 0, :n], wsl(1, 0, 3)[0:1, :],
                             ones[0:1, :n], start=False, stop=True,
                             tile_position=(0, 0))
            nc.tensor.matmul(ps[:, 1, :n], wsl(1, 1, 3)[32:33, :],
                             ones[32:33, :n], start=False, stop=True,
                             tile_position=(32, 0))
            post(ps, t, 1, n)

        def stage_l2(t):
            col, n = sched[t]
            prev = hid.pop((t, 1))
            hs = [prev[:, 0, :n], prev[:, 1, :n]]
            ps = pspool.tile([P, 2, 512], F32, tag="ps2")
            for m in range(2):
                for k in range(2):
                    nc.tensor.matmul(ps[:, m, :n], wsl(2, m, k), hs[k],
                                     start=(k == 0), stop=False)
            nc.tensor.matmul(ps[:, 0, :n], wsl(2, 0, 2)[0:1, :],
                             ones[0:1, :n], start=False, stop=True,
                             tile_position=(0, 0))
            nc.tensor.matmul(ps[:, 1, :n], wsl(2, 1, 2)[32:33, :],
                             ones[32:33, :n], start=False, stop=True,
                             tile_position=(32, 0))
            post(ps, t, 2, n)

        def stage_l3_l4(t3, t4):
            ps = ps4 = None
            if t3 is not None:
                _, n3 = sched[t3]
                prev = hid.pop((t3, 2))
                h2 = [prev[:, 0, :n3], prev[:, 1, :n3]]
                ps = pspool.tile([P, 2, 512], F32, tag="ps3")
                nc.tensor.matmul(ps[:, 0, :n3], wsl(3, 0, 0), h2[0],
                                 start=True, stop=False)
                nc.tensor.matmul(ps[:, 0, :n3], wsl(3, 0, 1), h2[1],
                                 start=False, stop=True)
            if t4 is not None:
                col4, n4 = sched[t4]
                h3 = hid.pop((t4, 3))
                ps4 = ps4pool.tile([1, 512], F32, tag="ps4")
            if t3 is not None:
                nc.tensor.matmul(ps[64:96, 1, :n3], wsl(3, 1, 0)[:, :32],
                                 h2[0], start=True, stop=False,
                                 tile_position=(0, 64))
            if t4 is not None:
                nc.tensor.matmul(ps4[:, :n4], wsl(4, 0, 0, width=1),
                                 h3[:, 0, :n4], start=True, stop=False,
                                 tile_position=(0, 0))
            if t3 is not None:
                nc.tensor.matmul(ps[64:96, 1, :n3], wsl(3, 1, 1)[:, :32],
                                 h2[1], start=False, stop=True,
                                 tile_position=(0, 64))
            if t4 is not None:
                nc.tensor.matmul(ps4[:, :n4], wsl(4, 0, 1, width=1)[64:96],
                                 h3[64:96, 1, :n4], start=False, stop=True,
                                 tile_position=(64, 0))
            if t3 is not None:
                post(ps, t3, 3, n3)
            if t4 is not None:
                nc.scalar.activation(ystage[:, col4:col4 + n4], ps4[:, :n4],
                                     AF.Identity, bias=bsb[0:1, 0:1],
                                     scale=1.0)
                nc.sync.dma_start(out_d[:, col4:col4 + n4],
                                  ystage[:, col4:col4 + n4])

        for t in range(-6, len(sched)):
            if 2 <= t + 7 < T:
                stage_load(t + 7)
            if t == -4:
                load_weights((2,))
            if t == -3:
                load_weights((3,))
            if t == -2:
                load_weights((4,))
            # during warmup, emit L1 before L2 so ready L1 matmuls aren't
            # stuck in the PE FIFO behind a waiting L2
            if t < 0 and 0 <= t + 4 < T:
                stage_l1(t + 4)
            if 0 <= t + 2 < T:
                stage_l2(t + 2)
            t3 = t + 1 if 0 <= t + 1 < T else None
            t4 = t if 0 <= t < T else None
            if t3 is not None or t4 is not None:
                stage_l3_l4(t3, t4)
            if t >= 0 and 0 <= t + 4 < T:
                stage_l1(t + 4)

    nc.compile()
    return nc


def kernel(**inputs):
    import ml_dtypes
    species = np.asarray(inputs["species"])
    out_dtype = np.asarray(inputs["aev"]).dtype
    aev_t, idx, sched, A_pc = _host_prepare(inputs)
    wimgs, bimgs = _fold_host(inputs)
    nc = _build_program(sched, A_pc)

    in_maps = [{"aev_t": np.ascontiguousarray(aev_t[c]),
                "wimg": wimgs[c // 2].astype(ml_dtypes.bfloat16),
                "bimg": bimgs[c // 2]}
               for c in range(NCORES)]
    res = run_bass_kernel_spmd(nc, in_maps, core_ids=list(range(NCORES)))

    out = np.zeros(species.size, dtype=np.float32)
    for c in range(NCORES):
        valid = idx[c] >= 0
        out[idx[c][valid]] = res.results[c]["out"][0][valid]
    return out.reshape(species.shape).astype(out_dtype, copy=False)
